# revision 25
# baseline (speedup 1.0000x reference)
"""2-layer GCN + FC on 8 trn2 NeuronCores.

Strategy (graph/data parallel, nodes sharded 12500/core by dst):
  out = relu(dinv_d * psum_d + Zpre_d) per GCN layer, where
  psum_d = sum_{e:dst=d} h_scaled[src_e], h_scaled = (x @ W) * dinv,
  Zpre = h_scaled * dinv + bias (uses dinv*sqrt(deg) == 1).

Per core: compute h_scaled for own 12.5k nodes; AllGather it in TWO halves
(first 49 blocks, rest) so the gather pipeline starts while the tail still
computes. The full table lives in a permuted order (half-major, then core)
split into 4 chunks < 32768 rows (int16 gather indices). Edges grouped by
(dst superblock of 8 blocks, src chunk): dma_gather h_scaled rows, slabs
of 128 edges may span dst blocks (pieces); one fused stride-0-broadcast
is_equal builds a whole group's one-hot pieces in fp8; TensorE
matmul-accumulates the segment sum in PSUM per dst block. Relu/scale
epilogues run on the idle Activation engine.
"""
import numpy as np

import concourse.bass as bass
import concourse.bacc as bacc
import concourse.mybir as mybir
import concourse.tile as tile
from concourse.bass_utils import run_bass_kernel_spmd
from concourse.masks import make_identity

NCORES = 8
N = 100000
NSHARD = N // NCORES          # 12500
P = 128
F = 128
FOUT = 64
NBLK = (NSHARD + P - 1) // P  # 98
NPAD = NBLK * P               # 12544
LAST_ROWS = NSHARD - (NBLK - 1) * P  # 84
BLK_A = 49                    # blocks in AllGather half A
ROWS_A = BLK_A * P            # 6272
ROWS_B = NSHARD - ROWS_A      # 6228
NA = NCORES * ROWS_A          # 50176 rows in table A
NB = NCORES * ROWS_B          # 49824 rows in table B
CHUNKS_P = [0, NA // 2, NA, NA + NB // 2, N]  # permuted-space chunks
NCHUNK = 4
SBSIZE = 4                    # dst blocks per superblock
NSB = (NBLK + SBSIZE - 1) // SBSIZE  # 13
MAXCALL_SLABS = 8             # 1024 indices per dma_gather (ring limit)

FP = mybir.dt.float32
BF = mybir.dt.bfloat16
F8 = mybir.dt.float8e4
I16 = mybir.dt.int16
I32 = mybir.dt.int32
NQ = 4  # SWDGE queues for gather issue overlap


def _permute(src):
    """Node id -> position in the two-half AllGather table order."""
    c = src // NSHARD
    l = src % NSHARD
    return np.where(l < ROWS_A, c * ROWS_A + l, NA + c * ROWS_B + (l - ROWS_A))


def _preprocess(edge_index):
    src = np.asarray(edge_index[0], dtype=np.int64)
    dst = np.asarray(edge_index[1], dtype=np.int64)
    deg = (np.bincount(dst, minlength=N) + 1.0).astype(np.float32)
    dinv = (1.0 / np.sqrt(deg)).astype(np.float32)

    pos = _permute(src)
    core = dst // NSHARD
    blk = (dst % NSHARD) // P
    sb = blk // SBSIZE
    chunkid = np.searchsorted(CHUNKS_P, pos, side="right") - 1
    gkey = (core * NSB + sb) * NCHUNK + chunkid
    order = np.lexsort((pos, blk, gkey))
    ep = pos[order]
    ed = dst[order]
    eb = blk[order]
    eg = gkey[order]
    ek = chunkid[order]

    ngroups = NCORES * NSB * NCHUNK
    cnt_g = np.bincount(eg, minlength=ngroups).reshape(NCORES, NSB, NCHUNK)
    nsl = np.ceil(cnt_g.max(axis=0) / P).astype(np.int64)  # [NSB, NCHUNK]

    # global slab layout: sb-major, then chunk
    gstart = np.zeros((NSB, NCHUNK), dtype=np.int64)
    sb_slab_start = np.zeros(NSB, dtype=np.int64)
    sb_nsl = np.zeros(NSB, dtype=np.int64)
    s_g = 0
    for sbi in range(NSB):
        sb_slab_start[sbi] = s_g
        for k in range(NCHUNK):
            gstart[sbi, k] = s_g
            s_g += nsl[sbi, k]
        sb_nsl[sbi] = s_g - sb_slab_start[sbi]
    S = s_g

    gofs = np.zeros(ngroups + 1, dtype=np.int64)
    np.cumsum(cnt_g.reshape(-1), out=gofs[1:])

    # union pieces per (sb, k): blocks present in each slab for any core
    pieces = []            # global list: (sbi, k, s_local, b)
    piece_base = {}        # (sbi, k) -> (first piece idx, npieces)
    blocks_of_sb = [list(range(b0, min(b0 + SBSIZE, NBLK)))
                    for b0 in range(0, NBLK, SBSIZE)]
    for sbi in range(NSB):
        blocks = blocks_of_sb[sbi]
        for k in range(NCHUNK):
            pbase = len(pieces)
            ns = int(nsl[sbi, k])
            if ns == 0:
                piece_base[(sbi, k)] = (pbase, 0)
                continue
            present = np.zeros((ns, len(blocks)), dtype=bool)
            for c in range(NCORES):
                g = (c * NSB + sbi) * NCHUNK + k
                lo, hi = gofs[g], gofs[g + 1]
                if hi == lo:
                    continue
                bcnt = np.bincount(eb[lo:hi] - blocks[0],
                                   minlength=len(blocks))
                ends = np.cumsum(bcnt)
                starts = ends - bcnt
                for bi in range(len(blocks)):
                    if bcnt[bi] == 0:
                        continue
                    s0, s1 = starts[bi] // P, (ends[bi] - 1) // P
                    present[s0:s1 + 1, bi] = True
            for s_local in range(ns):
                for bi in range(len(blocks)):
                    if present[s_local, bi]:
                        pieces.append((sbi, k, s_local, blocks[bi]))
            piece_base[(sbi, k)] = (pbase, len(pieces) - pbase)
    NPIECES = len(pieces)

    pieces_of_block = {}
    for pi, (sbi, k, s_local, b) in enumerate(pieces):
        pieces_of_block.setdefault(b, []).append(pi)

    # gather calls: per (sb, k), <=8 slabs each
    calls = []
    for sbi in range(NSB):
        for k in range(NCHUNK):
            ns = int(nsl[sbi, k])
            p0 = 0
            while p0 < ns:
                n = min(MAXCALL_SLABS, ns - p0)
                calls.append((sbi, k, int(gstart[sbi, k]) + p0, n))
                p0 += n

    piece_lut = {}
    for pi, (sbi, k, s_local, b) in enumerate(pieces):
        piece_lut[(sbi, k, s_local, b)] = pi

    # per-core payloads
    gidx_maps = []
    dstl_maps = []
    for c in range(NCORES):
        gidx16 = np.zeros((S, P), dtype=np.int16)
        dstl = np.full((NPIECES, P), -1.0, dtype=np.float32)
        for sbi in range(NSB):
            for k in range(NCHUNK):
                g = (c * NSB + sbi) * NCHUNK + k
                lo, hi = gofs[g], gofs[g + 1]
                n = hi - lo
                if n == 0:
                    continue
                posn = np.arange(n)
                sl = posn // P
                pp = posn % P
                slab_g = gstart[sbi, k] + sl
                gidx16[slab_g, pp] = (ep[lo:hi] - CHUNKS_P[k]).astype(np.int16)
                bloc = eb[lo:hi]
                dloc = (ed[lo:hi] % NSHARD) - bloc * P
                pid = np.array([piece_lut[(sbi, k, int(s), int(b))]
                                for s, b in zip(sl, bloc)], dtype=np.int64)
                dstl[pid, pp] = dloc.astype(np.float32)
        a = gidx16.reshape(S, 8, 16).transpose(2, 0, 1).reshape(16, S * 8)
        gidx_maps.append(np.tile(a, (8, 1)).copy())
        dstl_maps.append(dstl.T.copy())  # [P, NPIECES]

    return dict(
        deg=deg, dinv=dinv, S=S, NPIECES=NPIECES,
        nsl=nsl, gstart=gstart, sb_slab_start=sb_slab_start, sb_nsl=sb_nsl,
        pieces=pieces, piece_base=piece_base,
        pieces_of_block=pieces_of_block, calls=calls,
        blocks_of_sb=blocks_of_sb,
        gidx=gidx_maps, dstl=dstl_maps,
    )


def _build(meta):
    S = meta["S"]
    NPIECES = meta["NPIECES"]
    gstart = meta["gstart"]
    sb_slab_start = meta["sb_slab_start"]
    sb_nsl = meta["sb_nsl"]
    pieces = meta["pieces"]
    piece_base = meta["piece_base"]
    pieces_of_block = meta["pieces_of_block"]
    calls = meta["calls"]
    blocks_of_sb = meta["blocks_of_sb"]
    max_sb_slabs = int(max(sb_nsl))
    max_np = max(npieces for (_, npieces) in piece_base.values())

    nc = bacc.Bacc("TRN2", target_bir_lowering=False, debug=False,
                   num_devices=NCORES, num_swdge_queues=NQ,
                   dynamic_dma_scratch_size=32768)
    xT = nc.declare_dram_parameter("xT", [P, NPAD], BF, isOutput=False)
    w1 = nc.declare_dram_parameter("w1", [F, F], BF, isOutput=False)
    w2 = nc.declare_dram_parameter("w2", [F, F], BF, isOutput=False)
    wfc = nc.declare_dram_parameter("wfc", [F, FOUT], BF, isOutput=False)
    b1r = nc.declare_dram_parameter("b1r", [P, F], FP, isOutput=False)
    b2r = nc.declare_dram_parameter("b2r", [P, F], FP, isOutput=False)
    bfcr = nc.declare_dram_parameter("bfcr", [P, FOUT], FP, isOutput=False)
    dinv_p = nc.declare_dram_parameter("dinv", [P, NBLK], FP, isOutput=False)
    dinv2_p = nc.declare_dram_parameter("dinv2", [P, NBLK], FP, isOutput=False)
    gidx_p = nc.declare_dram_parameter("gidx", [P, S * 8], I16, isOutput=False)
    dstl_p = nc.declare_dram_parameter("dstl", [P, NPIECES], BF, isOutput=False)
    out_p = nc.declare_dram_parameter("out", [NSHARD, FOUT], FP, isOutput=True)

    with tile.TileContext(nc) as tc:
        with (
            tc.tile_pool(name="const", bufs=1) as cp,
            tc.tile_pool(name="xt", bufs=8) as xp,
            tc.tile_pool(name="hs", bufs=4) as hp,
            tc.tile_pool(name="z", bufs=100) as zp,
            tc.tile_pool(name="gbuf", bufs=3) as gp,
            tc.tile_pool(name="oh", bufs=8) as ohp,
            tc.tile_pool(name="rt", bufs=3) as rp,
            tc.tile_pool(name="relu", bufs=3) as lp,
            tc.tile_pool(name="outp", bufs=3) as op_,
            tc.tile_pool(name="pagg", bufs=3, space="PSUM") as pag,
            tc.tile_pool(name="pmisc", bufs=5, space="PSUM") as pms,
            tc.tile_pool(name="dram", bufs=1, space="DRAM") as dr,
        ):
            # ---- constants ----
            w1_t = cp.tile([F, F], BF); nc.sync.dma_start(out=w1_t[:], in_=w1[:])
            w2_t = cp.tile([F, F], BF); nc.sync.dma_start(out=w2_t[:], in_=w2[:])
            wfc_t = cp.tile([F, FOUT], BF); nc.sync.dma_start(out=wfc_t[:], in_=wfc[:])
            b1_t = cp.tile([P, F], FP); nc.sync.dma_start(out=b1_t[:], in_=b1r[:])
            b2_t = cp.tile([P, F], FP); nc.sync.dma_start(out=b2_t[:], in_=b2r[:])
            bfc_t = cp.tile([P, FOUT], FP); nc.sync.dma_start(out=bfc_t[:], in_=bfcr[:])
            dinv_t = cp.tile([P, NBLK], FP); nc.sync.dma_start(out=dinv_t[:], in_=dinv_p[:])
            dinv2_t = cp.tile([P, NBLK], FP); nc.sync.dma_start(out=dinv2_t[:], in_=dinv2_p[:])
            gidx_t = cp.tile([P, S * 8], I16); nc.sync.dma_start(out=gidx_t[:], in_=gidx_p[:])
            dstl_t = cp.tile([P, NPIECES], BF); nc.sync.dma_start(out=dstl_t[:], in_=dstl_p[:])
            iota_i = cp.tile([P, F], I32)
            nc.gpsimd.iota(iota_i[:], pattern=[[1, F]], base=0, channel_multiplier=0)
            iota_f = cp.tile([P, F], BF)
            nc.vector.tensor_copy(out=iota_f[:], in_=iota_i[:])
            ident = cp.tile([P, P], FP)
            make_identity(nc, ident[:])

            # ---- internal DRAM ----
            h1a_shard = dr.tile([ROWS_A, F], BF)
            h1b_shard = dr.tile([ROWS_B, F], BF)
            h2a_shard = dr.tile([ROWS_A, F], BF)
            h2b_shard = dr.tile([ROWS_B, F], BF)
            h1_full_a = dr.tile([NA, F], BF, addr_space="Shared")
            h1_full_b = dr.tile([NB, F], BF, addr_space="Shared")
            h2_full_a = dr.tile([NA, F], BF, addr_space="Shared")
            h2_full_b = dr.tile([NB, F], BF, addr_space="Shared")

            def rows_of(b):
                return LAST_ROWS if b == NBLK - 1 else P

            # h-store groups: [0..47] in 4s, [48], [49..96] in 4s, [97]
            HSG = 4
            hs_groups = ([(g, 4) for g in range(0, 48, 4)] + [(48, 1)]
                         + [(g, 4) for g in range(49, 93, 4)] + [(93, 4)]
                         + [(97, 1)])
            hs_group_of = {}
            for g0, nb in hs_groups:
                for j in range(nb):
                    hs_group_of[g0 + j] = (g0, nb, j)

            def flush_hs(g0, nb, gtile, ha, hb):
                rows = sum(rows_of(g0 + j) for j in range(nb))
                if g0 < BLK_A:
                    base, tgt = g0 * P, ha
                else:
                    base, tgt = (g0 - BLK_A) * P, hb
                tsl = tgt[:, :]
                out_ap = bass.AP(tsl.tensor, tsl.offset + base * F,
                                 [[F, P], [P * F, nb], [1, F]])
                if nb == 1:
                    out_ap = bass.AP(tsl.tensor, tsl.offset + base * F,
                                     [[F, rows_of(g0)], [1, F]])
                sl = gtile[:, :nb * F]
                in_ap = bass.AP(sl.tensor, sl.offset,
                                [[sl.ap[0][0], rows_of(g0) if nb == 1 else P],
                                 [1, nb * F]])
                nc.sync.dma_start(out=out_ap, in_=in_ap)

            def ag(ins_t, outs_t):
                nc.gpsimd.collective_compute(
                    "AllGather", mybir.AluOpType.bypass,
                    replica_groups=[list(range(NCORES))],
                    ins=[ins_t.opt()], outs=[outs_t.opt()])

            hs_cur = [None]

            def compute_h(b, lhsT_ap, w_t, b_t, ha, hb, z_tiles):
                """h_scaled + Zpre for one 128-node tile of own shard."""
                ps = pms.tile([P, F], FP, space="PSUM", tag="pm")
                nc.tensor.matmul(ps[:], lhsT=lhsT_ap, rhs=w_t[:], start=True, stop=True)
                g0, nb, j = hs_group_of[b]
                if j == 0:
                    hs_cur[0] = hp.tile([P, HSG * F], BF, name="hsg")
                hsg = hs_cur[0]
                nc.scalar.activation(
                    out=hsg[:, j * F:(j + 1) * F], in_=ps[:],
                    func=mybir.ActivationFunctionType.Copy,
                    scale=dinv_t[:, b:b + 1])
                if j == nb - 1:
                    flush_hs(g0, nb, hsg, ha, hb)
                zt = zp.tile([P, F], BF, tag="zt")
                nc.vector.scalar_tensor_tensor(
                    out=zt[:], in0=ps[:], scalar=dinv2_t[:, b:b + 1],
                    in1=b_t[:], op0=mybir.AluOpType.mult,
                    op1=mybir.AluOpType.add)
                z_tiles[b] = zt

            # ---- phase A: layer-1 h_scaled for own shard; split AllGather ----
            z1_tiles = {}
            z2_tiles = {}
            XB = 8
            for b0 in range(0, NBLK, XB):
                nb = min(XB, NBLK - b0)
                xt_t = xp.tile([P, XB * P], BF)
                nc.sync.dma_start(out=xt_t[:, :nb * P],
                                  in_=xT[:, b0 * P:(b0 + nb) * P])
                for j in range(nb):
                    b = b0 + j
                    compute_h(b, xt_t[:, j * P:(j + 1) * P], w1_t, b1_t,
                              h1a_shard, h1b_shard, z1_tiles)
                    if b == BLK_A - 1:
                        ag(h1a_shard, h1_full_a)
            ag(h1b_shard, h1_full_b)

            def agg_layer(tbl_a, tbl_b, z_tiles, z_out_tiles, layer,
                          ha=None, hb=None, fa=None, fb=None):
                """Edge aggregation; per block produce rT (transposed relu).

                layer==1: feed L2 compute, AllGather halves as they finish.
                """
                tables = [tbl_a[0:CHUNKS_P[1], :],
                          tbl_a[CHUNKS_P[1]:NA, :],
                          tbl_b[0:CHUNKS_P[3] - NA, :],
                          tbl_b[CHUNKS_P[3] - NA:NB, :]]
                qrr = [0]
                ot_cur = [None]
                for sbi in range(NSB):
                    blocks = blocks_of_sb[sbi]
                    sb_s = int(sb_slab_start[sbi])
                    gb = gp.tile([P, max_sb_slabs * F], BF, tag="gb")
                    for (csbi, k, s0, ns) in calls:
                        if csbi != sbi:
                            continue
                        o = s0 - sb_s
                        sl = gb[:, o * F:(o + ns) * F]
                        out_ap = bass.AP(sl.tensor, sl.offset,
                                         [sl.ap[0], [F, ns], [1, F]])
                        nc.gpsimd.dma_gather(
                            out_ap=out_ap,
                            in_ap=tables[k],
                            idxs_ap=gidx_t[:, s0 * 8:(s0 + ns) * 8],
                            num_idxs=ns * P,
                            num_idxs_reg=ns * P,
                            elem_size=F,
                            queue_num=qrr[0] % NQ,
                            single_packet=False,
                        )
                        qrr[0] += 1
                    oh_tiles = {}
                    for k in range(NCHUNK):
                        pbase, npieces = piece_base[(sbi, k)]
                        if npieces == 0:
                            continue
                        ohg = ohp.tile([P, max_np * F], F8, tag="oh")
                        in0 = bass.AP(iota_f.tensor, iota_f.offset,
                                      [iota_f.ap[0], [0, npieces], [1, F]])
                        dsl = dstl_t[:, pbase:pbase + npieces]
                        in1 = bass.AP(dsl.tensor, dsl.offset,
                                      [dsl.ap[0], [1, npieces], [0, F]])
                        nc.vector.tensor_tensor(
                            out=ohg[:, :npieces * F], in0=in0, in1=in1,
                            op=mybir.AluOpType.is_equal)
                        oh_tiles[k] = ohg
                    psum = pag.tile([P, SBSIZE * F], FP, space="PSUM", tag="pa")
                    for bi, b in enumerate(blocks):
                        plist = pieces_of_block.get(b, [])
                        for i, pi in enumerate(plist):
                            (psbi, k, s_local, _pb) = pieces[pi]
                            pbase, _ = piece_base[(sbi, k)]
                            po = pi - pbase
                            so = int(gstart[sbi, k]) - sb_s + s_local
                            nc.tensor.matmul(
                                psum[:, bi * F:(bi + 1) * F],
                                lhsT=oh_tiles[k][:, po * F:(po + 1) * F],
                                rhs=gb[:, so * F:(so + 1) * F],
                                start=(i == 0), stop=(i == len(plist) - 1))
                        # epilogue for block b
                        zt = z_tiles[b]
                        t = lp.tile([P, F], FP, tag="t1")
                        if plist:
                            nc.vector.scalar_tensor_tensor(
                                out=t[:], in0=psum[:, bi * F:(bi + 1) * F],
                                scalar=dinv_t[:, b:b + 1], in1=zt[:],
                                op0=mybir.AluOpType.mult,
                                op1=mybir.AluOpType.add)
                        else:
                            nc.vector.tensor_copy(out=t[:], in_=zt[:])
                        r = lp.tile([P, F], FP, tag="t2")
                        nc.scalar.activation(
                            out=r[:], in_=t[:],
                            func=mybir.ActivationFunctionType.Relu)
                        pst = pms.tile([P, F], FP, space="PSUM", tag="pm")
                        nc.tensor.transpose(out=pst[:], in_=r[:], identity=ident[:])
                        rT = rp.tile([P, F], BF)
                        nc.scalar.activation(
                            out=rT[:], in_=pst[:],
                            func=mybir.ActivationFunctionType.Copy)
                        if layer == 1:
                            compute_h(b, rT[:], w2_t, b2_t, ha, hb, z_out_tiles)
                            if b == BLK_A - 1:
                                ag(ha, fa)
                        else:
                            pfc = pms.tile([P, FOUT], FP, space="PSUM", tag="pm")
                            nc.tensor.matmul(pfc[:], lhsT=rT[:], rhs=wfc_t[:],
                                             start=True, stop=True)
                            oj = b % 4
                            if oj == 0:
                                ot_cur[0] = op_.tile([P, 4 * FOUT], FP, name="otg")
                            otg = ot_cur[0]
                            nc.vector.tensor_tensor(
                                out=otg[:, oj * FOUT:(oj + 1) * FOUT],
                                in0=pfc[:], in1=bfc_t[:],
                                op=mybir.AluOpType.add)
                            if oj == 3:
                                g0 = (b // 4) * 4
                                osl = out_p[:, :]
                                o_ap = bass.AP(
                                    osl.tensor, osl.offset + g0 * P * FOUT,
                                    [[FOUT, P], [P * FOUT, 4], [1, FOUT]])
                                sl = otg[:, :4 * FOUT]
                                i_ap = bass.AP(sl.tensor, sl.offset,
                                               [sl.ap[0], [1, 4 * FOUT]])
                                nc.sync.dma_start(out=o_ap, in_=i_ap)
                            elif b == NBLK - 1:
                                # last pair 96 (128 rows) + 97 (84 rows)
                                nc.sync.dma_start(
                                    out=out_p[96 * P:96 * P + P, :],
                                    in_=otg[:, 0:FOUT])
                                nc.sync.dma_start(
                                    out=out_p[97 * P:97 * P + LAST_ROWS, :],
                                    in_=otg[:LAST_ROWS, FOUT:2 * FOUT])
                if layer == 1:
                    ag(hb, fb)

            # ---- phase C: L1 aggregation + L2 compute + split AllGather 2 ----
            agg_layer(h1_full_a, h1_full_b, z1_tiles, z2_tiles, layer=1,
                      ha=h2a_shard, hb=h2b_shard, fa=h2_full_a, fb=h2_full_b)

            # ---- phase E: L2 aggregation + FC ----
            agg_layer(h2_full_a, h2_full_b, z2_tiles, None, layer=2)

    nc.compile()
    return nc


_CACHE = {}


def _get_nc(meta):
    key = (meta["S"], meta["NPIECES"], meta["nsl"].tobytes(),
           tuple(meta["pieces"]))
    key = hash(key)
    if key not in _CACHE:
        _CACHE[key] = _build(meta)
    return _CACHE[key]


def _pack_inputs(x, W1, b1, W2, b2, Wfc, bfc, meta):
    import ml_dtypes
    x = np.asarray(x, dtype=np.float32)
    dinv = meta["dinv"]
    in_maps = []
    for c in range(NCORES):
        lo = c * NSHARD
        xT = np.zeros((P, NPAD), dtype=np.float32)
        xT[:, :NSHARD] = x[lo:lo + NSHARD].T
        dv = np.ones(NPAD, dtype=np.float32)
        dv[:NSHARD] = dinv[lo:lo + NSHARD]
        in_maps.append({
            "xT": xT.astype(ml_dtypes.bfloat16),
            "w1": np.asarray(W1, dtype=np.float32).astype(ml_dtypes.bfloat16),
            "w2": np.asarray(W2, dtype=np.float32).astype(ml_dtypes.bfloat16),
            "wfc": np.asarray(Wfc, dtype=np.float32).astype(ml_dtypes.bfloat16),
            "b1r": np.tile(np.asarray(b1, dtype=np.float32)[None, :], (P, 1)),
            "b2r": np.tile(np.asarray(b2, dtype=np.float32)[None, :], (P, 1)),
            "bfcr": np.tile(np.asarray(bfc, dtype=np.float32)[None, :], (P, 1)),
            "dinv": dv.reshape(NBLK, P).T.copy(),
            "dinv2": (dv * dv).reshape(NBLK, P).T.copy(),
            "gidx": meta["gidx"][c],
            "dstl": meta["dstl"][c].astype(ml_dtypes.bfloat16),
        })
    return in_maps


def run(x, edge_index, W1, b1, W2, b2, Wfc, bfc, trace=False):
    meta = _preprocess(edge_index)
    nc = _get_nc(meta)
    in_maps = _pack_inputs(x, W1, b1, W2, b2, Wfc, bfc, meta)
    r = run_bass_kernel_spmd(nc, in_maps, list(range(NCORES)), trace=trace)
    out = np.concatenate([np.asarray(r.results[c]["out"]) for c in range(NCORES)], axis=0)
    return out.astype(np.float32), r


def kernel(**inputs):
    out, _ = run(**inputs)
    return out
